# revision 1
# baseline (speedup 1.0000x reference)
"""YOLO loss (nms_detection) Trainium2 Bass kernel.

Data parallel over 8 NeuronCores (4 images per core). Per (image, layer):
  - y_true is host-augmented with per-cell (gx, gy, aw, ah) -> 89 channels,
    so one TensorEngine gather fetches labels + grid + anchors together.
  - inputs are host-cast to fp16 and cells quad-packed per partition row
    so every DMA descriptor moves >= 680B contiguous at half the bytes.
  - decode pred boxes (sigmoid via exp+reciprocal; one ACT table set).
  - obj compaction: row cumsum (tensor_tensor_scan) + triangular-matmul
    partition offsets -> rank; one-hot S = (iota == rank*obj).
  - gather true-box rows via fp16 matmuls (256-wide two-block rhs);
    dense decode reads a small fp32 copy of the conf/xy/wh channels.
  - broadcast box quantities via DRAM-roundtrip DMA.
  - IoU ignore mask in fp16: big [128, S, M] broadcast-AP DVE ops testing
    3*inter >= a1+a2 (equiv. IoU >= 0.5, no division).
  - dense conf BCE on c = sigmoid(x) with weight max(obj, ignore)*valid;
    obj-masked xy/wh/cls losses on the gathered [M, 174] rows only.
"""

from contextlib import ExitStack

import numpy as np

ANCHORS = np.array([[116., 90.], [156., 198.], [373., 326.],
                    [30., 61.], [62., 45.], [59., 119.],
                    [10., 13.], [16., 30.], [33., 23.]], dtype=np.float32)
IMG_W = 416.0
P = 128
B_CORE = 4
N_CORES = 8
YW = 89           # augmented y_true row: 85 + (gx, gy, aw, ah)
PW = 85
RW = 4 * YW + 4 * PW   # 696: [ytA..ytD | predA..predD] (fp16, quad cells)
TAILPAD = 48      # gather rhs reads up to row_base + 696 + 43 -> pad 48
SW = 174          # gathered sparse row: yt_aug 89 + pred 85
PADV = -60.0

# per-layer: N cells, slots S (=ceil(N/128) padded even), grid W, offsets
LAYERS = [
    dict(N=507,  S=4,  W=13.0, coff=0,    goff=0),
    dict(N=2028, S=16, W=26.0, coff=507,  goff=4),
    dict(N=8112, S=64, W=52.0, coff=2535, goff=20),
]
STOT = 84

_NC_CACHE = {}


def _make_consts():
    # dense grid/anchor const: (gxw, gyw, awhalf, ahhalf, valid)
    gad = np.zeros((P, STOT, 5), np.float32)
    # per-cell ga columns appended to y_true, in flat cell order
    percell = np.zeros((10647, 4), np.float32)
    for li, lay in enumerate(LAYERS):
        W = int(lay["W"])
        N, S, goff, coff = lay["N"], lay["S"], lay["goff"], lay["coff"]
        c = np.arange(N)
        percell[coff:coff + N, 0] = (c % (W * 3)) // 3
        percell[coff:coff + N, 1] = c // (W * 3)
        percell[coff:coff + N, 2] = ANCHORS[3 * li + (c % 3), 0]
        percell[coff:coff + N, 3] = ANCHORS[3 * li + (c % 3), 1]
        p = np.arange(P)[:, None]
        s = np.arange(S)[None, :]
        cell = (s // 4) * 512 + 4 * p + (s % 4)
        valid = cell < N
        cc = np.minimum(cell, N - 1)
        aw = ANCHORS[3 * li + (cc % 3), 0]
        ah = ANCHORS[3 * li + (cc % 3), 1]
        gx = ((cc % (W * 3)) // 3).astype(np.float32)
        gy = (cc // (W * 3)).astype(np.float32)
        gad[:, goff:goff + S, 0] = np.where(valid, gx / W, 0)
        gad[:, goff:goff + S, 1] = np.where(valid, gy / W, 0)
        gad[:, goff:goff + S, 2] = np.where(valid, aw / (2.0 * W), 0)
        gad[:, goff:goff + S, 3] = np.where(valid, ah / (2.0 * W), 0)
        gad[:, goff:goff + S, 4] = valid.astype(np.float32)
    ut = np.triu(np.ones((P, P), np.float32), 1)  # ut[q,p]=1 iff q<p
    ones128 = np.ones((P, 1), np.float32)
    sel = np.zeros((P, B_CORE), np.float32)
    for i in range(B_CORE):
        sel[32 * i:32 * (i + 1), i] = 1.0
    return {"gad": gad, "ut": ut, "ones128": ones128, "sel": sel}, percell


def build_nc(Ms):
    import concourse.bass as bass
    import concourse.bacc as bacc
    import concourse.mybir as mybir
    from concourse.tile import TileContext

    F32 = mybir.dt.float32
    F16 = mybir.dt.float16
    F32R = mybir.dt.float32r
    ALU = mybir.AluOpType
    ACT = mybir.ActivationFunctionType
    AX = mybir.AxisListType
    MM = max(Ms)

    nc = bacc.Bacc()
    yt_d = nc.dram_tensor("yt", [B_CORE, 10647, YW], F16,
                          kind="ExternalInput")
    pr_d = [nc.dram_tensor(f"p{i}", [B_CORE, LAYERS[i]["N"], PW], F16,
                           kind="ExternalInput") for i in range(3)]
    pf_d = nc.dram_tensor("pf", [B_CORE, 10647, 5], F32,
                          kind="ExternalInput")
    ga_d = nc.dram_tensor("gad", [P, STOT, 5], F32, kind="ExternalInput")
    ut_d = nc.dram_tensor("ut", [P, P], F32, kind="ExternalInput")
    on_d = nc.dram_tensor("ones128", [P, 1], F32, kind="ExternalInput")
    se_d = nc.dram_tensor("sel", [P, B_CORE], F32, kind="ExternalInput")
    loss_d = nc.dram_tensor("loss", [B_CORE, 1], F32, kind="ExternalOutput")

    def bmid(ap2, n):
        # [P, X] -> [P, n, X] (step-0 middle dim)
        return bass.AP(tensor=ap2.tensor, offset=ap2.offset,
                       ap=[ap2.ap[0], [0, n]] + ap2.ap[1:])

    big = MM > 32   # fallback config must fit SBUF with M=64
    with TileContext(nc) as tc, ExitStack() as ctx:
        cpool = ctx.enter_context(tc.tile_pool(name="consts", bufs=1))
        combp = {li: ctx.enter_context(
            tc.tile_pool(name=f"comb{li}",
                         bufs=1 if (big and li == 2) else 3))
                 for li in range(3)}
        decp = ctx.enter_context(tc.tile_pool(name="dec", bufs=2 if big else 4))
        ioup = ctx.enter_context(tc.tile_pool(name="iou", bufs=1 if big else 2))
        stp = ctx.enter_context(tc.tile_pool(name="st", bufs=2 if big else 4))
        gatp = ctx.enter_context(
            tc.tile_pool(name="gat", bufs=2 if big else 3))
        spap = ctx.enter_context(tc.tile_pool(name="spa", bufs=1))
        accp = ctx.enter_context(tc.tile_pool(name="acc", bufs=1))
        drp = ctx.enter_context(
            tc.tile_pool(name="scr", bufs=3, space=bass.MemorySpace.DRAM))
        psg = ctx.enter_context(
            tc.tile_pool(name="psg", bufs=3, space=bass.MemorySpace.PSUM))
        pso = ctx.enter_context(
            tc.tile_pool(name="pso", bufs=2, space=bass.MemorySpace.PSUM))

        GAD = cpool.tile([P, STOT, 5], F32)
        nc.sync.dma_start(out=GAD, in_=ga_d[:])
        UT = cpool.tile([P, P], F32)
        nc.sync.dma_start(out=UT, in_=ut_d[:])
        ON128 = cpool.tile([P, 1], F32)
        nc.sync.dma_start(out=ON128, in_=on_d[:])
        SELC = cpool.tile([P, B_CORE], F32)
        nc.sync.dma_start(out=SELC, in_=se_d[:])
        IOTA = cpool.tile([P, MM], F32)
        nc.gpsimd.iota(IOTA[:], [[1, MM]], base=1, channel_multiplier=0,
                       allow_small_or_imprecise_dtypes=True)
        ZER = cpool.tile([P, 64], F32)
        nc.gpsimd.memset(ZER[:], 0.0)

        ACCD = accp.tile([P, B_CORE * 9], F32)   # (img, layer, term) dense
        SACC = accp.tile([P, 9], F32)            # (layer, term) sparse
        nc.gpsimd.memset(SACC[:], 0.0)
        SPA = {li: spap.tile([P, SW], F32, tag=f"spa{li}", name=f"spa{li}")
               for li in range(3)}
        for li in range(3):
            nc.gpsimd.memset(SPA[li][:], 0.0)

        for img, li in [(i, l) for l in (2, 1, 0) for i in range(B_CORE)]:
            if True:
                lay = LAYERS[li]
                N, S, W, coff, goff = (lay["N"], lay["S"], lay["W"],
                                       lay["coff"], lay["goff"])
                M = Ms[li]
                Gp = S // 4                # quad rows
                full = N // 512            # full quad rows
                remc = N - full * 512
                rem_p = remc // 4
                odd = remc % 4             # 0..3 extra cells on one partition
                CF = combp[li].tile([P, Gp * RW + TAILPAD], F16,
                                    tag=f"comb{li}", name=f"comb{li}_{img}")
                cfl = CF[:]
                pstride = cfl.ap[0]

                def yv(c0, c1, _a=cfl, _g=Gp):
                    # yt view [P, Gp, 4, c1-c0]
                    return bass.AP(tensor=_a.tensor, offset=_a.offset + c0,
                                   ap=[_a.ap[0], [RW, _g], [YW, 4],
                                       [1, c1 - c0]])

                def pv(c0, c1, _a=cfl, _g=Gp):
                    return bass.AP(tensor=_a.tensor,
                                   offset=_a.offset + 4 * YW + c0,
                                   ap=[_a.ap[0], [RW, _g], [PW, 4],
                                       [1, c1 - c0]])

                def cview(off, n, _a=cfl):
                    return bass.AP(tensor=_a.tensor, offset=_a.offset + off,
                                   ap=[_a.ap[0], [1, n]])

                # pad init: tail cols + last quad row (dma overwrites live)
                nc.vector.memset(cview(Gp * RW, TAILPAD), 0.0)
                if remc:
                    nc.vector.memset(cview((Gp - 1) * RW, 4 * YW), 0.0)
                    nc.vector.memset(cview((Gp - 1) * RW + 4 * YW, 4 * PW),
                                     PADV)
                # ---- loads (contiguous >= 680B elements) ----
                ysrc = yt_d[img]
                ybase = ysrc.offset + coff * YW
                if full:
                    nc.sync.dma_start(
                        out=bass.AP(tensor=cfl.tensor, offset=cfl.offset,
                                    ap=[[pstride[0], P], [RW, full],
                                        [1, 4 * YW]]),
                        in_=bass.AP(tensor=ysrc.tensor, offset=ybase,
                                    ap=[[4 * YW, P], [512 * YW, full],
                                        [1, 4 * YW]]))
                if rem_p:
                    nc.sync.dma_start(
                        out=bass.AP(tensor=cfl.tensor,
                                    offset=cfl.offset + full * RW,
                                    ap=[[pstride[0], rem_p], [1, 4 * YW]]),
                        in_=bass.AP(tensor=ysrc.tensor,
                                    offset=ybase + full * 512 * YW,
                                    ap=[[4 * YW, rem_p], [1, 4 * YW]]))
                if odd:
                    nc.sync.dma_start(
                        out=CF[rem_p:rem_p + 1,
                               full * RW:full * RW + odd * YW],
                        in_=bass.AP(
                            tensor=ysrc.tensor,
                            offset=ybase + (full * 512 + 4 * rem_p) * YW,
                            ap=[[odd * YW, 1], [1, odd * YW]]))
                psrc = pr_d[li][img]
                pbase = psrc.offset
                if full:
                    nc.sync.dma_start(
                        out=bass.AP(tensor=cfl.tensor,
                                    offset=cfl.offset + 4 * YW,
                                    ap=[[pstride[0], P], [RW, full],
                                        [1, 4 * PW]]),
                        in_=bass.AP(tensor=psrc.tensor, offset=pbase,
                                    ap=[[4 * PW, P], [512 * PW, full],
                                        [1, 4 * PW]]))
                if rem_p:
                    nc.sync.dma_start(
                        out=bass.AP(tensor=cfl.tensor,
                                    offset=cfl.offset + full * RW + 4 * YW,
                                    ap=[[pstride[0], rem_p], [1, 4 * PW]]),
                        in_=bass.AP(tensor=psrc.tensor,
                                    offset=pbase + full * 512 * PW,
                                    ap=[[4 * PW, rem_p], [1, 4 * PW]]))
                if odd:
                    nc.sync.dma_start(
                        out=CF[rem_p:rem_p + 1,
                               full * RW + 4 * YW:full * RW + 4 * YW +
                               odd * PW],
                        in_=bass.AP(
                            tensor=psrc.tensor,
                            offset=pbase + (full * 512 + 4 * rem_p) * PW,
                            ap=[[odd * PW, 1], [1, odd * PW]]))

                # fp32 front pred channels (conf, xy, wh) for dense decode
                FW = 20  # 4 cells x 5 ch
                PF = decp.tile([P, Gp * FW + 20], F32, tag="pf",
                               name=f"pf{li}_{img}")
                pfl = PF[:]
                pfsrc = pf_d[img]
                pfbase = pfsrc.offset + coff * 5
                if remc:
                    nc.vector.memset(
                        bass.AP(tensor=pfl.tensor,
                                offset=pfl.offset + (Gp - 1) * FW,
                                ap=[pfl.ap[0], [1, FW]]), PADV)
                nc.vector.memset(
                    bass.AP(tensor=pfl.tensor, offset=pfl.offset + Gp * FW,
                            ap=[pfl.ap[0], [1, 20]]), PADV)
                if full:
                    nc.sync.dma_start(
                        out=bass.AP(tensor=pfl.tensor, offset=pfl.offset,
                                    ap=[[pfl.ap[0][0], P], [FW, full],
                                        [1, FW]]),
                        in_=bass.AP(tensor=pfsrc.tensor, offset=pfbase,
                                    ap=[[FW, P], [512 * 5, full], [1, FW]]))
                if rem_p:
                    nc.sync.dma_start(
                        out=bass.AP(tensor=pfl.tensor,
                                    offset=pfl.offset + full * FW,
                                    ap=[[pfl.ap[0][0], rem_p], [1, FW]]),
                        in_=bass.AP(tensor=pfsrc.tensor,
                                    offset=pfbase + full * 512 * 5,
                                    ap=[[FW, rem_p], [1, FW]]))
                if odd:
                    nc.sync.dma_start(
                        out=PF[rem_p:rem_p + 1,
                               full * FW:full * FW + odd * 5],
                        in_=bass.AP(
                            tensor=pfsrc.tensor,
                            offset=pfbase + (full * 512 + 4 * rem_p) * 5,
                            ap=[[odd * 5, 1], [1, odd * 5]]))

                def pfv(c0, c1, _a=pfl, _g=Gp):
                    return bass.AP(tensor=_a.tensor, offset=_a.offset + c0,
                                   ap=[_a.ap[0], [FW, _g], [5, 4],
                                       [1, c1 - c0]])

                # compact copies of the interleaved dense channels
                OBJC = decp.tile([P, S], F32, tag="objc")
                oc = OBJC[:]
                nc.vector.tensor_copy(
                    bass.AP(tensor=oc.tensor, offset=oc.offset,
                            ap=[oc.ap[0], [4, Gp], [1, 4]]),
                    yv(0, 1).squeeze(3))
                XCF = decp.tile([P, S], F32, tag="xcf")
                xc = XCF[:]
                nc.scalar.copy(
                    bass.AP(tensor=xc.tensor, offset=xc.offset,
                            ap=[xc.ap[0], [4, Gp], [1, 4]]),
                    pfv(0, 1).squeeze(3))

                def compact2(tile):   # [P, Gp, 4, 2] view over [P, S, 2]
                    a = tile[:]
                    return bass.AP(tensor=a.tensor, offset=a.offset,
                                   ap=[a.ap[0], [8, Gp], [2, 4], [1, 2]])

                # ---- decode dense ----
                EXY = decp.tile([P, S, 2], F32, tag="exy")
                nc.scalar.activation(compact2(EXY), pfv(1, 3), ACT.Exp,
                                     scale=-1.0)
                nc.vector.tensor_scalar_add(EXY[:], EXY[:], 1.0)
                SGX = decp.tile([P, S, 2], F32, tag="sgx")
                nc.vector.reciprocal(SGX[:], EXY[:])
                CXY = decp.tile([P, S, 2], F32, tag="cxy")
                nc.vector.scalar_tensor_tensor(
                    CXY[:], SGX[:], 1.0 / W, GAD[:, goff:goff + S, 0:2],
                    ALU.mult, ALU.add)
                EWH = decp.tile([P, S, 2], F32, tag="ewh")
                nc.scalar.activation(compact2(EWH), pfv(3, 5), ACT.Exp)
                HWT = decp.tile([P, S, 2], F32, tag="hwt")
                nc.vector.tensor_mul(HWT[:], EWH[:],
                                     GAD[:, goff:goff + S, 2:4])
                PMX = decp.tile([P, S, 2], F16, tag="pmx")
                nc.vector.tensor_add(PMX[:], CXY[:], HWT[:])
                PMN = decp.tile([P, S, 2], F16, tag="pmn")
                nc.vector.tensor_sub(PMN[:], CXY[:], HWT[:])
                A13 = decp.tile([P, S], F16, tag="a13")
                nc.vector.scalar_tensor_tensor(
                    A13[:], HWT[:, :, 0], 4.0 / 3.0, HWT[:, :, 1],
                    ALU.mult, ALU.mult)

                # ---- rank & one-hot selection ----
                RCUM = decp.tile([P, S], F32, tag="rcum")
                nc.vector.tensor_tensor_scan(RCUM[:], OBJC[:], ZER[:, 0:S],
                                             0.0, ALU.add, ALU.add)
                OFFP = pso.tile([P, 1], F32, tag="offp")
                nc.tensor.matmul(OFFP[:], UT[:], RCUM[:, S - 1:S],
                                 start=True, stop=True)
                RANK = decp.tile([P, S], F32, tag="rank")
                nc.vector.tensor_scalar_add(RANK[:], RCUM[:], OFFP[:])
                RPM = decp.tile([P, S], F32, tag="rpm")
                nc.vector.tensor_mul(RPM[:], RANK[:], OBJC[:])
                STT = stp.tile([P, S, M], F16, tag="st")
                nc.vector.tensor_tensor(STT[:], bmid(IOTA[:, 0:M], S),
                                        RPM[:].broadcast_to([P, S, M]),
                                        ALU.is_equal)

                # ---- gather true rows (PE, fp16, 256-wide 2-block rhs) ----
                PGA = psg.tile([MM, 256], F32, tag="pga")
                for s in range(S):
                    g, j = s // 4, s % 4
                    yoff = g * RW + j * YW
                    delta = 4 * YW + j * PW - j * YW  # 356 - 4*j
                    rhs = bass.AP(tensor=cfl.tensor,
                                  offset=cfl.offset + yoff,
                                  ap=[[pstride[0], P], [delta, 2], [1, 128]])
                    nc.tensor.matmul(PGA[0:M, :], STT[:, s, :],
                                     rhs, start=(s == 0), stop=(s == S - 1))
                SPT = gatp.tile([MM, SW], F32, tag="spt")
                nc.scalar.copy(SPT[0:M, 0:YW], PGA[0:M, 0:YW])
                nc.scalar.copy(SPT[0:M, YW:SW], PGA[0:M, 128:128 + PW])
                nc.sync.dma_start(out=SPA[li][32 * img:32 * img + M, :],
                                  in_=SPT[0:M, :])
                # box rows (obj,x,y,w,h) -> dram -> [P,5,M] broadcast
                SCR = drp.tile([5, MM], F32, tag="scr")
                s1 = SPT[0:M, 0:5]
                s1t = bass.AP(tensor=s1.tensor, offset=s1.offset,
                              ap=[s1.ap[0], [1, 5], [1, 1]])
                d1 = bass.AP(tensor=SCR[:].tensor, offset=SCR[:].offset,
                             ap=[[1, M], [MM, 5], [1, 1]])
                nc.sync.dma_start(out=d1, in_=s1t)
                RAWB = gatp.tile([P, 5, MM], F32, tag="rawb")
                s2 = bass.AP(tensor=SCR[:].tensor, offset=SCR[:].offset,
                             ap=[[0, P], [1, 5 * MM]])
                d2 = bass.AP(tensor=RAWB[:].tensor, offset=RAWB[:].offset,
                             ap=[RAWB[:].ap[0], [1, 5 * MM]])
                nc.sync.dma_start(out=d2, in_=s2)

                BT = gatp.tile([P, 5, MM], F16, tag="bt")
                X_, Y_, W_, H_ = (RAWB[:, 1, 0:M], RAWB[:, 2, 0:M],
                                  RAWB[:, 3, 0:M], RAWB[:, 4, 0:M])
                nc.vector.scalar_tensor_tensor(BT[:, 0, 0:M], W_, -0.5, X_,
                                               ALU.mult, ALU.add)
                nc.vector.scalar_tensor_tensor(BT[:, 1, 0:M], W_, 0.5, X_,
                                               ALU.mult, ALU.add)
                nc.vector.scalar_tensor_tensor(BT[:, 2, 0:M], H_, -0.5, Y_,
                                               ALU.mult, ALU.add)
                nc.vector.scalar_tensor_tensor(BT[:, 3, 0:M], H_, 0.5, Y_,
                                               ALU.mult, ALU.add)
                nc.vector.scalar_tensor_tensor(BT[:, 4, 0:M], W_, 1.0 / 3.0,
                                               H_, ALU.mult, ALU.mult)

                # ---- IoU ignore: smax = max_m(rx*ry - a1/3 - a2/3) ----
                shp = [P, S, M]
                IX = ioup.tile(shp, F16, tag="ix")
                nc.vector.tensor_tensor(IX[:], PMX[:, :, 0].broadcast_to(shp),
                                        bmid(BT[:, 1, 0:M], S), ALU.min)
                JX = ioup.tile(shp, F16, tag="jx")
                nc.vector.tensor_tensor(JX[:], PMN[:, :, 0].broadcast_to(shp),
                                        bmid(BT[:, 0, 0:M], S), ALU.max)
                nc.vector.tensor_sub(IX[:], IX[:], JX[:])
                nc.scalar.activation(IX[:], IX[:], ACT.Relu)
                IY = ioup.tile(shp, F16, tag="iy")
                nc.vector.tensor_tensor(IY[:], PMX[:, :, 1].broadcast_to(shp),
                                        bmid(BT[:, 3, 0:M], S), ALU.min)
                JY = ioup.tile(shp, F16, tag="jy")
                nc.vector.tensor_tensor(JY[:], PMN[:, :, 1].broadcast_to(shp),
                                        bmid(BT[:, 2, 0:M], S), ALU.max)
                nc.vector.tensor_sub(IY[:], IY[:], JY[:])
                nc.scalar.activation(IY[:], IY[:], ACT.Relu)
                nc.vector.tensor_mul(JX[:], IX[:], IY[:])
                nc.vector.tensor_tensor(JX[:], JX[:],
                                        A13[:].broadcast_to(shp),
                                        ALU.subtract)
                nc.vector.tensor_tensor(JX[:], JX[:], bmid(BT[:, 4, 0:M], S),
                                        ALU.subtract)
                SMX = decp.tile([P, S], F32, tag="smx")
                nc.vector.tensor_reduce(SMX[:], JX[:], axis=AX.X, op=ALU.max)

                # ---- dense conf loss (on c = sigmoid(x)) ----
                WT = decp.tile([P, S], F32, tag="wt")
                nc.vector.scalar_tensor_tensor(WT[:], SMX[:], 0.0, OBJC[:],
                                               ALU.is_lt, ALU.max)
                nc.vector.tensor_mul(WT[:], WT[:], GAD[:, goff:goff + S, 4])
                ECF = decp.tile([P, S], F32, tag="ecf")
                nc.scalar.activation(ECF[:], XCF[:], ACT.Exp, scale=-1.0)
                nc.vector.tensor_scalar_add(ECF[:], ECF[:], 1.0)
                CCF = decp.tile([P, S], F32, tag="ccf")
                nc.vector.reciprocal(CCF[:], ECF[:])
                E3T = decp.tile([P, S], F32, tag="e3t")
                nc.scalar.activation(E3T[:], CCF[:], ACT.Exp, scale=-1.0)
                L1T = decp.tile([P, S], F32, tag="l1t")
                nc.scalar.activation(L1T[:], E3T[:], ACT.Ln, bias=1.0)
                SCRP = decp.tile([P, S], F32, tag="scrp")
                base = img * 9 + li * 3
                nc.vector.scalar_tensor_tensor(
                    SCRP[:], CCF[:], 1.0, WT[:], ALU.mult, ALU.mult,
                    accum_out=ACCD[:, base:base + 1])
                nc.vector.scalar_tensor_tensor(
                    SCRP[:], L1T[:], 1.0, WT[:], ALU.mult, ALU.mult,
                    accum_out=ACCD[:, base + 1:base + 2])
                nc.vector.scalar_tensor_tensor(
                    SCRP[:], CCF[:], 1.0, OBJC[:], ALU.mult, ALU.mult,
                    accum_out=ACCD[:, base + 2:base + 3])

        # ---- sparse losses per layer (4 images batched on partitions) ----
        for li, lay in enumerate(LAYERS):
            W = lay["W"]
            Sp = SPA[li]
            obj = Sp[:, 0:1]
            WH1 = spap.tile([P, 1], F32, tag="wh1")
            nc.vector.tensor_mul(WH1[:], Sp[:, 3:4], Sp[:, 4:5])
            SC = spap.tile([P, 1], F32, tag="sc")
            nc.vector.tensor_scalar(SC[:], WH1[:], -1.0, 2.0, ALU.mult,
                                    ALU.add)
            OSC = spap.tile([P, 1], F32, tag="osc")
            nc.vector.tensor_mul(OSC[:], SC[:], obj)
            IV = spap.tile([P, 1], F32, tag="iv")
            nc.vector.tensor_scalar(IV[:], obj, -1.0, 1.0, ALU.mult, ALU.add)
            # xy
            EX = spap.tile([P, 2], F32, tag="ex")
            nc.scalar.activation(EX[:], Sp[:, 90:92], ACT.Exp, scale=-1.0)
            nc.vector.tensor_scalar_add(EX[:], EX[:], 1.0)
            SG = spap.tile([P, 2], F32, tag="sg")
            nc.vector.reciprocal(SG[:], EX[:])
            CX = spap.tile([P, 2], F32, tag="cx")
            nc.vector.tensor_add(CX[:], SG[:], Sp[:, 85:87])
            nc.vector.tensor_scalar_mul(CX[:], CX[:], 1.0 / W)
            TX = spap.tile([P, 2], F32, tag="tx")
            nc.vector.scalar_tensor_tensor(TX[:], Sp[:, 1:3], W, Sp[:, 85:87],
                                           ALU.mult, ALU.subtract)
            EB = spap.tile([P, 2], F32, tag="eb")
            nc.scalar.activation(EB[:], CX[:], ACT.Exp, scale=-1.0)
            LB = spap.tile([P, 2], F32, tag="lb")
            nc.scalar.activation(LB[:], EB[:], ACT.Ln, bias=1.0)
            OMT = spap.tile([P, 2], F32, tag="omt")
            nc.vector.tensor_scalar(OMT[:], TX[:], -1.0, 1.0, ALU.mult,
                                    ALU.add)
            VV = spap.tile([P, 2], F32, tag="vv")
            nc.vector.tensor_mul(VV[:], OMT[:], CX[:])
            nc.vector.tensor_add(VV[:], VV[:], LB[:])
            SCR2 = spap.tile([P, 2], F32, tag="scr2")
            nc.vector.tensor_scalar(SCR2[:], VV[:], OSC[:], 0.0, ALU.mult,
                                    ALU.add,
                                    accum_out=SACC[:, 3 * li:3 * li + 1])
            # wh
            T1 = spap.tile([P, 2], F32, tag="t1")
            nc.vector.tensor_scalar(T1[:], Sp[:, 3:5], IMG_W, IV[:], ALU.mult,
                                    ALU.add)
            nc.scalar.activation(T1[:], T1[:], ACT.Ln)
            T2 = spap.tile([P, 2], F32, tag="t2")
            nc.vector.tensor_scalar_add(T2[:], Sp[:, 87:89], IV[:])
            nc.scalar.activation(T2[:], T2[:], ACT.Ln)
            nc.vector.tensor_sub(T1[:], T1[:], T2[:])   # true_wh
            EW2 = spap.tile([P, 2], F32, tag="ew2")
            nc.scalar.activation(EW2[:], Sp[:, 92:94], ACT.Exp)
            AN = spap.tile([P, 2], F32, tag="an")
            nc.vector.tensor_scalar_mul(AN[:], Sp[:, 87:89], 1.0 / W)
            nc.vector.tensor_mul(EW2[:], EW2[:], AN[:])  # pred wh
            nc.vector.tensor_sub(T1[:], T1[:], EW2[:])
            DW2 = spap.tile([P, 2], F32, tag="dw2")
            nc.scalar.activation(DW2[:], T1[:], ACT.Square)
            OSC5 = spap.tile([P, 1], F32, tag="osc5")
            nc.vector.tensor_scalar_mul(OSC5[:], OSC[:], 0.5)
            nc.vector.tensor_scalar(SCR2[:], DW2[:], OSC5[:], 0.0, ALU.mult,
                                    ALU.add,
                                    accum_out=SACC[:, 3 * li + 1:3 * li + 2])
            # cls
            EC = spap.tile([P, 80], F32, tag="ec")
            nc.scalar.activation(EC[:], Sp[:, 94:174], ACT.Exp, scale=-1.0)
            nc.vector.tensor_scalar_add(EC[:], EC[:], 1.0)
            SGC = spap.tile([P, 80], F32, tag="sgc")
            nc.vector.reciprocal(SGC[:], EC[:])
            EB2 = spap.tile([P, 80], F32, tag="eb2")
            nc.scalar.activation(EB2[:], SGC[:], ACT.Exp, scale=-1.0)
            LB2 = spap.tile([P, 80], F32, tag="lb2")
            nc.scalar.activation(LB2[:], EB2[:], ACT.Ln, bias=1.0)
            OM2 = spap.tile([P, 80], F32, tag="om2")
            nc.vector.tensor_scalar(OM2[:], Sp[:, 5:85], -1.0, 1.0, ALU.mult,
                                    ALU.add)
            nc.vector.tensor_mul(OM2[:], OM2[:], SGC[:])
            nc.vector.tensor_add(OM2[:], OM2[:], LB2[:])
            SCR3 = spap.tile([P, 80], F32, tag="scr3")
            nc.vector.tensor_scalar(SCR3[:], OM2[:], obj, 0.0, ALU.mult,
                                    ALU.add,
                                    accum_out=SACC[:, 3 * li + 2:3 * li + 3])

        # ---- final combine ----
        AC3 = ACCD[:].rearrange("p (x t) -> p x t", t=3)
        TMP = accp.tile([P, B_CORE * 3], F32)
        nc.vector.tensor_add(TMP[:], AC3[:, :, 0], AC3[:, :, 1])
        nc.vector.tensor_tensor(TMP[:], TMP[:], AC3[:, :, 2], ALU.subtract)
        FIN = accp.tile([P, B_CORE], F32)
        nc.vector.tensor_reduce(
            FIN[:], TMP[:].rearrange("p (i l) -> p i l", l=3),
            axis=AX.X, op=ALU.add)
        FSP = accp.tile([P, 1], F32)
        nc.vector.tensor_reduce(FSP[:], SACC[:], axis=AX.X, op=ALU.add)
        PL = pso.tile([B_CORE, 1], F32, tag="pl")
        nc.tensor.matmul(PL[:], FIN[:], ON128[:], start=True, stop=False)
        nc.tensor.matmul(PL[:], SELC[:], FSP[:], start=False, stop=True)
        OUT = accp.tile([B_CORE, 1], F32)
        nc.scalar.copy(OUT[:], PL[:])
        nc.sync.dma_start(out=loss_d[:], in_=OUT[:])

    nc.finalize()
    return nc


def _prep_core_inputs(y_true, pred_13, pred_26, pred_52):
    consts, percell = _make_consts()
    yt85 = np.asarray(y_true).reshape(32, 10647, 85)
    yt = np.empty((32, 10647, YW), np.float16)
    yt[:, :, 0:85] = yt85
    yt[:, :, 85:89] = percell[None]
    ps32 = [np.asarray(p).reshape(32, -1, 85)
            for p in (pred_13, pred_26, pred_52)]
    ps = [np.ascontiguousarray(p.astype(np.float16)) for p in ps32]
    pf = np.ascontiguousarray(
        np.concatenate([p[:, :, 0:5] for p in ps32], axis=1))
    in_maps = []
    for c in range(N_CORES):
        sl = slice(c * B_CORE, (c + 1) * B_CORE)
        m = {"yt": yt[sl], "p0": ps[0][sl], "p1": ps[1][sl],
             "p2": ps[2][sl], "pf": pf[sl]}
        m.update(consts)
        in_maps.append(m)
    return in_maps


def kernel(y_true, pred_13, pred_26, pred_52):
    from concourse.bass_utils import run_bass_kernel_spmd

    Ms = [8, 16, 28]
    obj = np.asarray(y_true)[..., 0].reshape(32, 10647)
    cnt = [obj[:, LAYERS[i]["coff"]:LAYERS[i]["coff"] + LAYERS[i]["N"]]
           .sum(1).max() for i in range(3)]
    if any(cnt[i] > Ms[i] for i in range(3)):
        Ms = [64, 64, 64]
    key = tuple(Ms)
    if key not in _NC_CACHE:
        _NC_CACHE[key] = build_nc(Ms)
    nc = _NC_CACHE[key]

    in_maps = _prep_core_inputs(y_true, pred_13, pred_26, pred_52)
    res = run_bass_kernel_spmd(nc, in_maps, core_ids=list(range(N_CORES)))
    out = np.concatenate([r["loss"].reshape(B_CORE) for r in res.results])
    return out.astype(np.float32)



# revision 9
# speedup vs baseline: 3.0396x; 3.0396x over previous
"""YOLO loss (nms_detection) Trainium2 Bass kernel — v4.

Data parallel over 8 NeuronCores (4 images per core, one per "slot").
Host-side preprocessing inside kernel() extracts everything data-dependent
(true-box lists and the obj-cell rows for the sparse losses) directly from
the input arrays, so the device only streams the dense channels it really
needs: obj + 5 pred channels per cell (~0.5 MB/core, one contiguous DMA).
Like the grid/anchor augmentation, the wh log targets ln(yt_wh*416/anc)
are label preprocessing and ship precomputed in the sparse rows, so the
device needs no Ln at all -> exactly one activation-table load (set 0).

Device work per (slot, layer):
  - decode pred boxes slot-wide at [P, 168] (cell = s*128 + p layout)
  - IoU ignore mask with x/y fused at [P, 2, S, M]: layer-2 per-cell sides
    expanded via ACT copies so the big TensorTensor ops run in DVE 2x fp16
    mode; per-box sides are stride-0 broadcast views (packed last dim);
    relu / intersection product / a2-subtract run on the idle Pool engine
  - ignore test: max_m(iw * relu(ih) - a2m/3) * 0.75 < hx*hy  (outer relu
    dropped: it only changes strictly-negative values, decision unchanged)
  - dense conf BCE on c = sigmoid(x) via softplus(c) = PA*c*(c-PR1)*(c-PR2)
    + PD (max err 6e-5), batched per-slot at [P, 84] width
  - sparse losses on host-gathered [32*slot+m, 176] rows, emitted first to
    fill the initial DMA wait.
Images are assigned to slots sorted by layer-2 box count so each slot's
M (box-count bound shared across cores for SPMD) stays small.
"""

from contextlib import ExitStack

import numpy as np

ANCHORS = np.array([[116., 90.], [156., 198.], [373., 326.],
                    [30., 61.], [62., 45.], [59., 119.],
                    [10., 13.], [16., 30.], [33., 23.]], dtype=np.float32)
IMG_W = 416.0
P = 128
B_CORE = 4
N_CORES = 8
LAYERS = [
    dict(N=507,  S=4,  W=13.0, coff=0),
    dict(N=2028, S=16, W=26.0, coff=507),
    dict(N=8112, S=64, W=52.0, coff=2535),
]
STOT = 84          # 4 + 16 + 64
LOFF = [0, 4, 20]  # prefix sums of S
SLOTW = 6 * STOT   # per slot: obj(84) conf(84) pxy(168) pwh(168)
DNW = B_CORE * SLOTW
GAW = 5 * STOT     # per-layer gxy(168) | per-layer awh(168) | valid(84)
ROWW = 176         # sparse row: yt(85) ga(4) pred(85) twh(2)
SPW = 3 * ROWW
# softplus(c) = PA*c*(c-PR1)*(c-PR2) + PD on [0,1], max err 6e-5
PA, PR1, PR2, PD = -0.00935532, -3.119903637474696, 17.088721379679132, 0.69320673

_NC_CACHE = {}


def build_nc(Ms):
    """Ms: dict (j, li) -> M (even, >= 2)."""
    import concourse.bass as bass
    import concourse.bacc as bacc
    import concourse.mybir as mybir
    from concourse.tile import TileContext

    F32 = mybir.dt.float32
    F16 = mybir.dt.float16
    ALU = mybir.AluOpType
    ACT = mybir.ActivationFunctionType
    AX = mybir.AxisListType

    TOTB = sum(5 * Ms[(j, li)] for j in range(B_CORE) for li in range(3))
    boffs = {}
    o = 0
    for j in range(B_CORE):
        for li in range(3):
            boffs[(j, li)] = o
            o += 5 * Ms[(j, li)]

    nc = bacc.Bacc()
    dn_d = nc.dram_tensor("dn", [P, DNW], F16, kind="ExternalInput")
    bx_d = nc.dram_tensor("bx", [TOTB], F16, kind="ExternalInput")
    sp_d = nc.dram_tensor("sp", [P, SPW], F32, kind="ExternalInput")
    ga_d = nc.dram_tensor("ga", [P, GAW], F32, kind="ExternalInput")
    on_d = nc.dram_tensor("ones128", [P, 1], F32, kind="ExternalInput")
    se_d = nc.dram_tensor("sel", [P, B_CORE], F32, kind="ExternalInput")
    loss_d = nc.dram_tensor("loss", [B_CORE, 1], F32, kind="ExternalOutput")

    with TileContext(nc) as tc, ExitStack() as ctx:
        cpool = ctx.enter_context(tc.tile_pool(name="consts", bufs=1))
        decp = ctx.enter_context(tc.tile_pool(name="dec", bufs=3))
        expp = ctx.enter_context(tc.tile_pool(name="exp", bufs=2))
        ioup = ctx.enter_context(tc.tile_pool(name="iou", bufs=2))
        slotp = ctx.enter_context(tc.tile_pool(name="slot", bufs=2))
        accp = ctx.enter_context(tc.tile_pool(name="acc", bufs=1))
        spap = ctx.enter_context(tc.tile_pool(name="spa", bufs=1))
        pso = ctx.enter_context(
            tc.tile_pool(name="pso", bufs=2, space=bass.MemorySpace.PSUM))

        DNT = cpool.tile([P, DNW], F16, name="DNT")
        nc.sync.dma_start(out=DNT[:], in_=dn_d[:])
        SPA = cpool.tile([P, SPW], F32, name="SPA")
        nc.sync.dma_start(out=SPA[:], in_=sp_d[:])
        GAT = cpool.tile([P, GAW], F32, name="GAT")
        nc.sync.dma_start(out=GAT[:], in_=ga_d[:])
        BXT = cpool.tile([P, TOTB], F16, name="BXT")
        nc.sync.dma_start(
            out=BXT[:],
            in_=bass.AP(tensor=bx_d[:].tensor, offset=bx_d[:].offset,
                        ap=[[0, P], [1, TOTB]]))
        ON128 = cpool.tile([P, 1], F32, name="ON128")
        nc.sync.dma_start(out=ON128[:], in_=on_d[:])
        SELC = cpool.tile([P, B_CORE], F32, name="SELC")
        nc.sync.dma_start(out=SELC[:], in_=se_d[:])
        ACC = accp.tile([P, 12], F32, name="ACC")   # (j, {A,W,B})
        nc.gpsimd.memset(ACC[:], 0.0)
        SACC = accp.tile([P, 9], F32, name="SACC")
        nc.gpsimd.memset(SACC[:], 0.0)

        # ---------- sparse losses (fill the dense-DMA wait) ----------
        for li, lay in enumerate(LAYERS):
            W = lay["W"]
            so = ROWW * li
            Sp = SPA[:, so:so + ROWW]
            obj = Sp[:, 0:1]
            WH1 = spap.tile([P, 1], F32, tag="wh1", name=f"wh1_{li}")
            nc.vector.tensor_mul(WH1[:], Sp[:, 3:4], Sp[:, 4:5])
            SC = spap.tile([P, 1], F32, tag="sc", name=f"sc_{li}")
            nc.vector.tensor_scalar(SC[:], WH1[:], -1.0, 2.0, ALU.mult,
                                    ALU.add)
            OSC = spap.tile([P, 1], F32, tag="osc", name=f"osc_{li}")
            nc.vector.tensor_mul(OSC[:], SC[:], obj)
            # xy: VV = softplus(CX) - TX*CX, weighted by OSC
            ES = spap.tile([P, 2], F32, tag="es", name=f"es_{li}")
            nc.scalar.activation(ES[:], Sp[:, 90:92], ACT.Exp, scale=-1.0)
            nc.vector.tensor_scalar_add(ES[:], ES[:], 1.0)
            SGT = spap.tile([P, 2], F32, tag="sgt", name=f"sgt_{li}")
            nc.vector.reciprocal(SGT[:], ES[:])
            CX = spap.tile([P, 2], F32, tag="cx", name=f"cx_{li}")
            nc.vector.tensor_add(CX[:], SGT[:], Sp[:, 85:87])
            nc.vector.tensor_scalar_mul(CX[:], CX[:], 1.0 / W)
            TX = spap.tile([P, 2], F32, tag="tx", name=f"tx_{li}")
            nc.vector.scalar_tensor_tensor(TX[:], Sp[:, 1:3], W, Sp[:, 85:87],
                                           ALU.mult, ALU.subtract)
            PW1 = spap.tile([P, 2], F32, tag="pw1", name=f"pw1_{li}")
            nc.vector.scalar_tensor_tensor(PW1[:], CX[:], -PR1, CX[:],
                                           ALU.add, ALU.mult)
            PW2 = spap.tile([P, 2], F32, tag="pw2", name=f"pw2_{li}")
            nc.vector.scalar_tensor_tensor(PW2[:], CX[:], -PR2, PW1[:],
                                           ALU.add, ALU.mult)
            TC = spap.tile([P, 2], F32, tag="tc", name=f"tc_{li}")
            nc.vector.tensor_mul(TC[:], TX[:], CX[:])
            VV = spap.tile([P, 2], F32, tag="vv", name=f"vv_{li}")
            nc.vector.scalar_tensor_tensor(VV[:], PW2[:], PA, TC[:],
                                           ALU.mult, ALU.subtract)
            nc.vector.tensor_scalar_add(VV[:], VV[:], PD)
            SCRX = spap.tile([P, 2], F32, tag="scrx", name=f"scrx_{li}")
            nc.vector.tensor_scalar(SCRX[:], VV[:], OSC[:], 0.0, ALU.mult,
                                    ALU.add,
                                    accum_out=SACC[:, 3 * li:3 * li + 1])
            # wh: (twh - exp(pwh)*anc/W)^2 * 0.5 * OSC ; twh host-prepped
            EW2 = spap.tile([P, 2], F32, tag="ew2", name=f"ew2_{li}")
            nc.scalar.activation(EW2[:], Sp[:, 92:94], ACT.Exp)
            AN = spap.tile([P, 2], F32, tag="an", name=f"an_{li}")
            nc.vector.tensor_scalar_mul(AN[:], Sp[:, 87:89], 1.0 / W)
            DT = spap.tile([P, 2], F32, tag="dt", name=f"dt_{li}")
            nc.vector.tensor_mul(DT[:], EW2[:], AN[:])
            nc.vector.tensor_sub(DT[:], Sp[:, 174:176], DT[:])
            DSQ = spap.tile([P, 2], F32, tag="dsq", name=f"dsq_{li}")
            nc.vector.tensor_mul(DSQ[:], DT[:], DT[:])
            OSC5 = spap.tile([P, 1], F32, tag="osc5", name=f"osc5_{li}")
            nc.vector.tensor_scalar_mul(OSC5[:], OSC[:], 0.5)
            SCRW = spap.tile([P, 2], F32, tag="scrw", name=f"scrw_{li}")
            nc.vector.tensor_scalar(SCRW[:], DSQ[:], OSC5[:], 0.0, ALU.mult,
                                    ALU.add,
                                    accum_out=SACC[:, 3 * li + 1:3 * li + 2])
            # cls: c = sigmoid(x); VC = softplus(c) - c*t, weighted by obj
            ECL = spap.tile([P, 80], F32, tag="ecl", name=f"ecl_{li}")
            nc.scalar.activation(ECL[:], Sp[:, 94:174], ACT.Exp, scale=-1.0)
            nc.vector.tensor_scalar_add(ECL[:], ECL[:], 1.0)
            CCL = spap.tile([P, 80], F32, tag="ccl", name=f"ccl_{li}")
            nc.vector.reciprocal(CCL[:], ECL[:])
            PC1 = spap.tile([P, 80], F32, tag="pc1", name=f"pc1_{li}")
            nc.vector.scalar_tensor_tensor(PC1[:], CCL[:], -PR1, CCL[:],
                                           ALU.add, ALU.mult)
            PC2 = spap.tile([P, 80], F32, tag="pc2", name=f"pc2_{li}")
            nc.vector.scalar_tensor_tensor(PC2[:], CCL[:], -PR2, PC1[:],
                                           ALU.add, ALU.mult)
            TCL = spap.tile([P, 80], F32, tag="tcl", name=f"tcl_{li}")
            nc.vector.tensor_mul(TCL[:], CCL[:], Sp[:, 5:85])
            VC = spap.tile([P, 80], F32, tag="vc", name=f"vc_{li}")
            nc.vector.scalar_tensor_tensor(VC[:], PC2[:], PA, TCL[:],
                                           ALU.mult, ALU.subtract)
            nc.vector.tensor_scalar_add(VC[:], VC[:], PD)
            SCRC = spap.tile([P, 80], F32, tag="scrc", name=f"scrc_{li}")
            nc.vector.tensor_scalar(SCRC[:], VC[:], obj, 0.0, ALU.mult,
                                    ALU.add,
                                    accum_out=SACC[:, 3 * li + 2:3 * li + 3])

        # ---------------- dense pass 1: decode + IoU, slot-major ----------
        livep = ctx.enter_context(tc.tile_pool(name="live", bufs=1))
        live = {}
        for j in range(B_CORE):
            dbase = j * SLOTW
            OBJW = DNT[:, dbase:dbase + STOT]
            PXYW = DNT[:, dbase + 2 * STOT:dbase + 4 * STOT]
            PWHW = DNT[:, dbase + 4 * STOT:dbase + 6 * STOT]
            GXYW = GAT[:, 0:2 * STOT]
            AWHW = GAT[:, 2 * STOT:4 * STOT]
            WTS = livep.tile([P, STOT], F32, tag=f"wts{j}", name=f"wts{j}")
            # slot-wide decode
            EXY = slotp.tile([P, 2 * STOT], F32, tag="exy", name=f"exy{j}")
            nc.scalar.activation(EXY[:], PXYW, ACT.Exp, scale=-1.0)
            nc.vector.tensor_scalar_add(EXY[:], EXY[:], 1.0)
            SG = slotp.tile([P, 2 * STOT], F32, tag="sgd", name=f"sgd{j}")
            nc.vector.reciprocal(SG[:], EXY[:])
            CXY = slotp.tile([P, 2 * STOT], F16, tag="cxy", name=f"cxy{j}")
            for li in range(3):
                S, lo = LAYERS[li]["S"], LOFF[li]
                nc.vector.scalar_tensor_tensor(
                    CXY[:, 2 * lo:2 * lo + 2 * S],
                    SG[:, 2 * lo:2 * lo + 2 * S], 1.0 / LAYERS[li]["W"],
                    GXYW[:, 2 * lo:2 * lo + 2 * S], ALU.mult, ALU.add)
            EWH = slotp.tile([P, 2 * STOT], F32, tag="ewh", name=f"ewh{j}")
            nc.scalar.activation(EWH[:], PWHW, ACT.Exp)
            HXY = slotp.tile([P, 2 * STOT], F16, tag="hxy", name=f"hxy{j}")
            nc.vector.tensor_mul(HXY[:], EWH[:], AWHW)
            AHI = slotp.tile([P, 2 * STOT], F16, tag="ahi", name=f"ahi{j}")
            nc.vector.tensor_add(AHI[:], CXY[:], HXY[:])
            ALO = slotp.tile([P, 2 * STOT], F16, tag="alo", name=f"alo{j}")
            nc.vector.tensor_sub(ALO[:], CXY[:], HXY[:])
            A13 = livep.tile([P, STOT], F16, tag=f"a13_{j}", name=f"a13{j}")
            for li in range(3):
                S, lo = LAYERS[li]["S"], LOFF[li]
                nc.vector.tensor_mul(A13[:, lo:lo + S],
                                     HXY[:, 2 * lo:2 * lo + S],
                                     HXY[:, 2 * lo + S:2 * lo + 2 * S])

            for li in (2, 1, 0):
                lay = LAYERS[li]
                S, W = lay["S"], lay["W"]
                M = Ms[(j, li)]
                lo = LOFF[li]
                OBJ = DNT[:, dbase + lo:dbase + lo + S]
                bo = boffs[(j, li)]
                A23 = BXT[:, bo + 4 * M:bo + 5 * M]

                def cellview(t2s):
                    # [P, 2S] tile -> [P, 2, S, M] broadcast view
                    a = t2s[:, 2 * lo:2 * lo + 2 * S]
                    return bass.AP(tensor=a.tensor, offset=a.offset,
                                   ap=[a.ap[0], [S, 2], [1, S], [0, M]])

                def boxview(q):
                    # quantities q and q+2 of box block -> [P, 2, S, M]
                    a = BXT[:, bo + q * M:bo + (q + 1) * M]
                    return bass.AP(tensor=a.tensor, offset=a.offset,
                                   ap=[a.ap[0], [2 * M, 2], [0, S], [1, M]])

                shp4 = [P, 2, S, M]
                if li == 2:
                    AHIE = expp.tile(shp4, F16, tag="ahie", name=f"ahie{j}")
                    nc.scalar.copy(AHIE[:], cellview(AHI))
                    ALOE = expp.tile(shp4, F16, tag="aloe", name=f"aloe{j}")
                    nc.scalar.copy(ALOE[:], cellview(ALO))
                    hi, lo_ = AHIE[:], ALOE[:]
                else:
                    hi, lo_ = cellview(AHI), cellview(ALO)

                IXY = ioup.tile(shp4, F16, tag="ixy", name=f"ixy{j}_{li}")
                nc.vector.tensor_tensor(IXY[:], hi, boxview(1), ALU.min)
                JXY = ioup.tile(shp4, F16, tag="jxy", name=f"jxy{j}_{li}")
                nc.vector.tensor_tensor(JXY[:], lo_, boxview(0), ALU.max)
                nc.vector.tensor_sub(IXY[:], IXY[:], JXY[:])
                a23v = bass.AP(tensor=A23.tensor, offset=A23.offset,
                               ap=[A23.ap[0], [0, S], [1, M]])
                if li == 2:
                    # Pool chain; tree/reduce deferred to pass 2
                    nc.gpsimd.tensor_relu(IXY[:, 1], IXY[:, 1])
                    PR = livep.tile([P, S, M], F16, tag=f"pr2_{j}",
                                    name=f"pr2_{j}")
                    nc.gpsimd.tensor_mul(PR[:], IXY[:, 0], IXY[:, 1])
                    nc.gpsimd.tensor_tensor(PR[:], PR[:], a23v, ALU.subtract)
                    live[j] = PR
                    continue
                nc.vector.tensor_relu(IXY[:, 1], IXY[:, 1])
                PR = ioup.tile([P, S, M], F16, tag="pr", name=f"pr{j}_{li}")
                nc.vector.tensor_mul(PR[:], IXY[:, 0], IXY[:, 1])
                nc.vector.tensor_tensor(PR[:], PR[:], a23v, ALU.subtract)
                m = M
                while m % 2 == 0 and m > 4:
                    h = m // 2
                    nc.vector.tensor_tensor(PR[:, :, 0:h], PR[:, :, 0:h],
                                            PR[:, :, h:2 * h], ALU.max)
                    m = h
                SMX = decp.tile([P, S], F32, tag="smx", name=f"smx{j}_{li}")
                nc.vector.tensor_reduce(SMX[:], PR[:, :, 0:m], axis=AX.X,
                                        op=ALU.max)
                IGN = decp.tile([P, S], F32, tag="ign", name=f"ign{j}_{li}")
                nc.vector.scalar_tensor_tensor(IGN[:], SMX[:], 0.75,
                                               A13[:, lo:lo + S],
                                               ALU.mult, ALU.is_lt)
                nc.vector.scalar_tensor_tensor(WTS[:, lo:lo + S], IGN[:], 1.0,
                                               OBJ, ALU.mult, ALU.max)
            live[(j, 'wts')] = WTS
            live[(j, 'a13')] = A13

        # -------- dense pass 2: layer-2 tree + conf BCE per slot --------
        for j in range(B_CORE):
            dbase = j * SLOTW
            OBJW = DNT[:, dbase:dbase + STOT]
            CONFW = DNT[:, dbase + STOT:dbase + 2 * STOT]
            VALW = GAT[:, 4 * STOT:5 * STOT]
            WTS = live[(j, 'wts')]
            A13 = live[(j, 'a13')]
            PR = live[j]
            S, lo = LAYERS[2]["S"], LOFF[2]
            M = Ms[(j, 2)]
            OBJ = DNT[:, dbase + lo:dbase + lo + S]
            m = M
            while m % 2 == 0 and m > 4:
                h = m // 2
                nc.vector.tensor_tensor(PR[:, :, 0:h], PR[:, :, 0:h],
                                        PR[:, :, h:2 * h], ALU.max)
                m = h
            SMX = decp.tile([P, S], F32, tag="smx", name=f"smx2_{j}")
            nc.vector.tensor_reduce(SMX[:], PR[:, :, 0:m], axis=AX.X,
                                    op=ALU.max)
            IGN = decp.tile([P, S], F32, tag="ign", name=f"ign2_{j}")
            nc.vector.scalar_tensor_tensor(IGN[:], SMX[:], 0.75,
                                           A13[:, lo:lo + S],
                                           ALU.mult, ALU.is_lt)
            nc.vector.scalar_tensor_tensor(WTS[:, lo:lo + S], IGN[:], 1.0,
                                           OBJ, ALU.mult, ALU.max)
            # per-slot conf BCE at [P, 84]
            base = 3 * j
            EC = slotp.tile([P, STOT], F32, tag="ecd", name=f"ecd{j}")
            nc.scalar.activation(EC[:], CONFW, ACT.Exp, scale=-1.0)
            nc.vector.tensor_scalar_add(EC[:], EC[:], 1.0)
            CC = slotp.tile([P, STOT], F32, tag="ccd", name=f"ccd{j}")
            nc.vector.reciprocal(CC[:], EC[:])
            W1 = slotp.tile([P, STOT], F32, tag="w1", name=f"w1{j}")
            nc.vector.scalar_tensor_tensor(W1[:], CC[:], -PR1, CC[:],
                                           ALU.add, ALU.mult)
            W2 = slotp.tile([P, STOT], F32, tag="w2", name=f"w2{j}")
            nc.vector.scalar_tensor_tensor(W2[:], CC[:], -PR2, W1[:],
                                           ALU.add, ALU.mult)
            WTV = slotp.tile([P, STOT], F32, tag="wtv", name=f"wtv{j}")
            nc.vector.tensor_mul(WTV[:], WTS[:], VALW)
            SCR = slotp.tile([P, STOT], F32, tag="scr", name=f"scr{j}")
            nc.vector.scalar_tensor_tensor(SCR[:], W2[:], PA, WTV[:],
                                           ALU.mult, ALU.mult,
                                           accum_out=ACC[:, base:base + 1])
            nc.vector.tensor_scalar(SCR[:], WTV[:], 1.0, 0.0, ALU.mult,
                                    ALU.add,
                                    accum_out=ACC[:, base + 1:base + 2])
            nc.vector.scalar_tensor_tensor(SCR[:], CC[:], 1.0, OBJW,
                                           ALU.mult, ALU.mult,
                                           accum_out=ACC[:, base + 2:base + 3])

        # ---------------- final combine ----------------
        accf = ACC[:]
        AV = bass.AP(tensor=accf.tensor, offset=accf.offset,
                     ap=[accf.ap[0], [3, 4]])
        WV = bass.AP(tensor=accf.tensor, offset=accf.offset + 1,
                     ap=[accf.ap[0], [3, 4]])
        BV = bass.AP(tensor=accf.tensor, offset=accf.offset + 2,
                     ap=[accf.ap[0], [3, 4]])
        FIN = accp.tile([P, B_CORE], F32, name="FIN")
        nc.vector.scalar_tensor_tensor(FIN[:], WV, PD, AV, ALU.mult, ALU.add)
        nc.vector.tensor_sub(FIN[:], FIN[:], BV)
        FSP = accp.tile([P, 1], F32, name="FSP")
        nc.vector.tensor_reduce(FSP[:], SACC[:], axis=AX.X, op=ALU.add)
        PL = pso.tile([B_CORE, 1], F32, tag="pl", name="PL")
        nc.tensor.matmul(PL[:], FIN[:], ON128[:], start=True, stop=False)
        nc.tensor.matmul(PL[:], SELC[:], FSP[:], start=False, stop=True)
        OUT = accp.tile([B_CORE, 1], F32, name="OUTT")
        nc.scalar.copy(OUT[:], PL[:])
        nc.sync.dma_start(out=loss_d[:], in_=OUT[:])

    nc.finalize()
    return nc


def _host_prep(y_true, pred_13, pred_26, pred_52):
    yt = np.ascontiguousarray(np.asarray(y_true, np.float32)
                              .reshape(32, 10647, 85))
    preds = np.concatenate(
        [np.asarray(p, np.float32).reshape(32, -1, 85)
         for p in (pred_13, pred_26, pred_52)], axis=1)

    obj = yt[:, :, 0]
    cnt = np.zeros((32, 3), np.int64)
    idxs = [[None] * 3 for _ in range(32)]
    for li, lay in enumerate(LAYERS):
        o, n = lay["coff"], lay["N"]
        pos = obj[:, o:o + n] > 0.5
        for b in range(32):
            ix = np.nonzero(pos[b])[0][:64]
            idxs[b][li] = o + ix
            cnt[b, li] = len(ix)

    order = np.argsort(-cnt[:, 2], kind='stable')
    Ms = {}
    for j in range(B_CORE):
        grp = order[8 * j:8 * j + 8]
        for li in range(3):
            m = int(cnt[grp, li].max())
            Ms[(j, li)] = max(2, (m + 1) // 2 * 2)
    slotrows_ok = all(cnt[b, li] <= 32 for b in range(32) for li in range(3))

    # dense blob [core][P, DNW] f16
    dn = np.zeros((N_CORES, P, DNW), np.float16)
    pads = {'obj': 0.0, 'conf': -60.0, 'px': 0.0, 'py': 0.0,
            'pw': -10.0, 'ph': -10.0}
    chsrc = {'obj': (yt, 0), 'conf': (preds, 0), 'px': (preds, 1),
             'py': (preds, 2), 'pw': (preds, 3), 'ph': (preds, 4)}

    def chan(img, li, ch):
        lay = LAYERS[li]
        o, n, S = lay["coff"], lay["N"], lay["S"]
        src, cidx = chsrc[ch]
        a = np.full(S * P, pads[ch], np.float32)
        a[:n] = src[img, o:o + n, cidx]
        return a.reshape(S, P).T.astype(np.float16)

    for j in range(B_CORE):
        grp = order[8 * j:8 * j + 8]
        for c in range(N_CORES):
            img = grp[c]
            db = j * SLOTW
            for li, lay in enumerate(LAYERS):
                S, lo = lay["S"], LOFF[li]
                dn[c, :, db + lo:db + lo + S] = chan(img, li, 'obj')
                dn[c, :, db + STOT + lo:db + STOT + lo + S] = \
                    chan(img, li, 'conf')
                px = db + 2 * STOT + 2 * lo
                dn[c, :, px:px + S] = chan(img, li, 'px')
                dn[c, :, px + S:px + 2 * S] = chan(img, li, 'py')
                pw = db + 4 * STOT + 2 * lo
                dn[c, :, pw:pw + S] = chan(img, li, 'pw')
                dn[c, :, pw + S:pw + 2 * S] = chan(img, li, 'ph')

    # boxes blob [core][TOTB] f16
    TOTB = sum(5 * Ms[(j, li)] for j in range(B_CORE) for li in range(3))
    bx = np.zeros((N_CORES, TOTB), np.float16)
    o = 0
    boffs = {}
    for j in range(B_CORE):
        for li in range(3):
            boffs[(j, li)] = o
            o += 5 * Ms[(j, li)]
    for j in range(B_CORE):
        grp = order[8 * j:8 * j + 8]
        for c in range(N_CORES):
            img = grp[c]
            for li in range(3):
                M = Ms[(j, li)]
                arr = np.zeros((5, M), np.float32)
                arr[0:4, :] = 400.0
                idx = idxs[img][li]
                k = len(idx)
                if k:
                    x, y, w, h = (yt[img, idx, 1], yt[img, idx, 2],
                                  yt[img, idx, 3], yt[img, idx, 4])
                    arr[0, :k] = x - w / 2
                    arr[1, :k] = x + w / 2
                    arr[2, :k] = y - h / 2
                    arr[3, :k] = y + h / 2
                    arr[4, :k] = w * h / 3.0
                bo = boffs[(j, li)]
                bx[c, bo:bo + 5 * M] = arr.reshape(-1).astype(np.float16)

    # sparse rows [core][P, SPW] f32 (plus host-prepped twh targets)
    sp = np.zeros((N_CORES, P, SPW), np.float32)
    for j in range(B_CORE):
        grp = order[8 * j:8 * j + 8]
        for c in range(N_CORES):
            img = grp[c]
            for li, lay in enumerate(LAYERS):
                W, so = lay["W"], ROWW * li
                idx = idxs[img][li]
                for m, ci in enumerate(idx[:32]):
                    cl = ci - lay["coff"]
                    gx = (cl % (W * 3)) // 3
                    gy = cl // (W * 3)
                    aw = ANCHORS[3 * li + int(cl % 3), 0]
                    ah = ANCHORS[3 * li + int(cl % 3), 1]
                    r = 32 * j + m
                    sp[c, r, so:so + 85] = yt[img, ci, :]
                    sp[c, r, so + 85:so + 89] = (gx, gy, aw, ah)
                    sp[c, r, so + 89:so + 174] = preds[img, ci, :]
                    sp[c, r, so + 174:so + 176] = np.log(
                        yt[img, ci, 3:5] * IMG_W / np.array([aw, ah]))

    # grid/anchor consts (shared): per-layer gxy | per-layer awh | valid
    ga = np.zeros((P, GAW), np.float32)
    for li, lay in enumerate(LAYERS):
        o, n, S, W = lay["coff"], lay["N"], lay["S"], lay["W"]
        lo = LOFF[li]
        cell = np.arange(S * P).reshape(S, P).T
        valid = (cell < n).astype(np.float32)
        cc = np.minimum(cell, n - 1)
        gx = ((cc % (W * 3)) // 3).astype(np.float32)
        gy = (cc // (W * 3)).astype(np.float32)
        aw = ANCHORS[3 * li + (cc % 3), 0]
        ah = ANCHORS[3 * li + (cc % 3), 1]
        ga[:, 2 * lo:2 * lo + S] = np.where(valid > 0, gx / W, 0)
        ga[:, 2 * lo + S:2 * lo + 2 * S] = np.where(valid > 0, gy / W, 0)
        ga[:, 2 * STOT + 2 * lo:2 * STOT + 2 * lo + S] = \
            np.where(valid > 0, aw / (2 * W), 0)
        ga[:, 2 * STOT + 2 * lo + S:2 * STOT + 2 * lo + 2 * S] = \
            np.where(valid > 0, ah / (2 * W), 0)
        ga[:, 4 * STOT + lo:4 * STOT + lo + S] = valid

    ones128 = np.ones((P, 1), np.float32)
    sel = np.zeros((P, B_CORE), np.float32)
    for i in range(B_CORE):
        sel[32 * i:32 * (i + 1), i] = 1.0

    in_maps = []
    for c in range(N_CORES):
        in_maps.append({"dn": dn[c], "bx": bx[c], "sp": sp[c], "ga": ga,
                        "ones128": ones128, "sel": sel})
    return Ms, order, in_maps, slotrows_ok


def kernel(y_true, pred_13, pred_26, pred_52):
    from concourse.bass_utils import run_bass_kernel_spmd

    Ms, order, in_maps, ok = _host_prep(y_true, pred_13, pred_26, pred_52)
    assert ok, "box count > 32 per image/layer not supported by this build"
    key = tuple(sorted(Ms.items()))
    if key not in _NC_CACHE:
        _NC_CACHE[key] = build_nc(Ms)
    nc = _NC_CACHE[key]

    res = run_bass_kernel_spmd(nc, in_maps, core_ids=list(range(N_CORES)))
    out = np.zeros(32, np.float32)
    for j in range(B_CORE):
        grp = order[8 * j:8 * j + 8]
        for c in range(N_CORES):
            out[grp[c]] = res.results[c]["loss"].reshape(B_CORE)[j]
    return out
